# revision 36
# baseline (speedup 1.0000x reference)
"""Trainium2 Bass kernel for nn_Att_AdaIn (B=4, C=256, H=W=64 attention block).

Sharding: 8 cores = 4 batches x 2 query-halves. Each core holds the fused
weights, the full key/value source y[b] ([256, 4096]), and its own query
slice x[b][:, half] ([256, 2048]); it computes the full attention output for
its 2048 queries. Host gathers the 8 [256, 2048] results.

Weight fusion done on the host (in float64):
  logits: S = k^T q with q = Wq x + bq, k = Wk y + bk
        = y^T (Wk^T Wq) x + y^T (Wk^T bq) 1^T + [per-query-constant terms]
    The per-query-constant (l-only) terms are softmax-invariant and dropped.
    So with  M^T = (Wk^T Wq)^T  and  bw = Wk^T bq:   qm = M x + bw,
    ST[j,l] = sum_c y[c,j] qm[c,l].
  output: Wo (V E / den) + bo  with V = Wv y + bv 1^T
        = (Wo Wv) y E / den + Wo bv + bo
    So with MoT = (Wo Wv)^T and bo2 = bo + Wo bv, the value projection
    vTo = y^T MoT directly produces Wo-mixed values and the separate
    output projection disappears.

Per-core pipeline (layouts chosen so no on-chip transpose is needed):
  qm  = M x + bw               [c, l]      (c on partitions)
  vTo = y^T MoT                [j, 256]    (j on partitions)
  ST  = y^T qm                 [j, l]      (transposed attention scores)
  E   = exp(ST / sqrt(C))      (no max-subtraction: logits ~ N(0,1), fp32-safe)
  zq  = vTo^T E                [256, l]    unnormalized Wo-mixed output
  den = 1^T E                  [l]         softmax denominators (E summed on
                                           DVE, partition-reduced by one matmul)
  out = zq * (1/den) + bo2 + x

Dtype config via env:
  ATT_MATMUL_DT: float32 | float32r | bfloat16 (base matmul dtype)
  ATT_FP8: 0 = off (default; safest numerics, ~162 us),
           1 = fp8(e4m3) DoubleRow score matmuls (~130 us, rel err ~2e-3),
           2 = level 1 + fp8 DoubleRow qm/vTo projections,
           3 = level 2 + fp8 values/denominator with paired exps and a
               -2.5 logit shift (~127 us, rel err ~4e-3).
"""

import os
import sys

for _p in ("/root/.axon_site", "/root/.axon_site/_ro/trn_rl_repo", "/opt/trn_rl_repo"):
    if os.path.isdir(_p) and _p not in sys.path:
        sys.path.append(_p)

import numpy as np

import concourse.bass as bass
from concourse import bacc, mybir, tile
from concourse import bass_utils

B, C, H, W = 4, 256, 64, 64
N = H * W          # 4096 pixels
NQ = N // 2        # 2048 queries per core
P = 128
A = C // P         # 2 channel chunks
LT = 512           # l-tile (query) width
NLT = NQ // LT     # 4 l-tiles
JC = N // P        # 32 key chunks
SCALE = 1.0 / np.sqrt(np.float32(C))  # 1/16

MATMUL_DT = os.environ.get("ATT_MATMUL_DT", "bfloat16")
FP8_LEVEL = int(os.environ.get("ATT_FP8", "0"))


def build_nc(matmul_dt_name: str = MATMUL_DT, fp8_level: int = FP8_LEVEL):
    mdt = getattr(mybir.dt, matmul_dt_name)
    f32 = mybir.dt.float32
    f8 = mybir.dt.float8e4
    is_bf16 = mdt == mybir.dt.bfloat16
    st_fp8 = fp8_level >= 1 and is_bf16
    proj_fp8 = fp8_level >= 2 and is_bf16
    val_fp8 = fp8_level >= 3 and is_bf16
    DR = mybir.MatmulPerfMode.DoubleRow

    nc = bacc.Bacc("TRN2", target_bir_lowering=False, debug=False)

    # --- DRAM tensors ---
    xdt = f8 if proj_fp8 else mdt
    x_d = nc.dram_tensor("x", [C, NQ], xdt, kind="ExternalInput").ap()
    mT_d = nc.dram_tensor("mT", [C, C], xdt, kind="ExternalInput").ap()
    if st_fp8:
        y8_d = nc.dram_tensor("y8", [C, N], f8, kind="ExternalInput").ap()
    if not proj_fp8:
        y_d = nc.dram_tensor("y", [C, N], mdt, kind="ExternalInput").ap()
    moTa_d = nc.dram_tensor("moTa", [C, C], xdt, kind="ExternalInput").ap()
    bw_d = nc.dram_tensor("bw", [C], f32, kind="ExternalInput").ap()
    bo2_d = nc.dram_tensor("bo2", [C], f32, kind="ExternalInput").ap()
    if is_bf16:
        xres_d = nc.dram_tensor("xres", [C, NQ], f32, kind="ExternalInput").ap()
    out_d = nc.dram_tensor("out", [C, NQ], f32, kind="ExternalOutput").ap()

    qm_dt = f8 if st_fp8 else mdt

    with tile.TileContext(nc) as tc:
        with (
            tc.tile_pool(name="const", bufs=1) as const,
            tc.tile_pool(name="epool", bufs=8) as epool,
            tc.tile_pool(name="opool", bufs=3) as opool,
            tc.tile_pool(name="rpool", bufs=2) as rpool,
            tc.tile_pool(name="ps_st", bufs=2 if val_fp8 else 4, space="PSUM") as ps_st,
            tc.tile_pool(name="ps_zq", bufs=1 if val_fp8 else 2, space="PSUM") as ps_zq,
            tc.tile_pool(name="ps_small", bufs=1, space="PSUM") as ps_small,
            tc.tile_pool(name="dpool", bufs=2, space="DRAM") as dpool,
        ):
            # ---- persistent SBUF tensors ----
            x_sb = const.tile([P, A, NQ], xdt)
            mT_sb = const.tile([P, A, C], xdt)
            if st_fp8:
                y8_sb = const.tile([P, A, N], f8)
            if not proj_fp8:
                y_sb = const.tile([P, A, N], mdt)
            moTa_sb = const.tile([P, A, C], xdt)
            bw_sb = const.tile([P, A], f32)
            bo2_sb = const.tile([P, A], f32)
            ones_col = const.tile([P, 1], mdt)
            ones_row = const.tile([1, P], mdt)
            ones_p2 = const.tile([P, 2, 16], f8)
            shift_sb = const.tile([P, 1], f32)
            qm_sb = const.tile([P, A, NQ], qm_dt)
            vTo_sb = const.tile([P, JC, C], f8 if val_fp8 else mdt)
            if is_bf16:
                xres_sb = const.tile([P, A, NQ], f32)
            else:
                xres_sb = x_sb.bitcast(f32)

            # ---- loads (in order of first use; xres last, needed only at the end) ----
            xr_ = x_d.rearrange("(a p) n -> p a n", p=P)
            nc.sync.dma_start(out=x_sb[:, :, :NQ // 2], in_=xr_[:, :, :NQ // 2])
            nc.sync.dma_start(out=mT_sb, in_=mT_d.rearrange("(a p) o -> p a o", p=P))
            nc.sync.dma_start(out=bw_sb, in_=bw_d.rearrange("(a p) -> p a", p=P))
            nc.sync.dma_start(out=x_sb[:, :, NQ // 2:], in_=xr_[:, :, NQ // 2:])
            if not proj_fp8:
                yr_ = y_d.rearrange("(a p) n -> p a n", p=P)
                nc.sync.dma_start(out=y_sb[:, :, :N // 2], in_=yr_[:, :, :N // 2])
            nc.sync.dma_start(out=moTa_sb, in_=moTa_d.rearrange("(a p) o -> p a o", p=P))
            if not proj_fp8:
                nc.sync.dma_start(out=y_sb[:, :, N // 2:], in_=yr_[:, :, N // 2:])
            if st_fp8:
                y8r_ = y8_d.rearrange("(a p) n -> p a n", p=P)
                nc.sync.dma_start(out=y8_sb[:, :, :N // 2], in_=y8r_[:, :, :N // 2])
                nc.sync.dma_start(out=y8_sb[:, :, N // 2:], in_=y8r_[:, :, N // 2:])
            nc.sync.dma_start(out=bo2_sb, in_=bo2_d.rearrange("(a p) -> p a", p=P))
            nc.vector.memset(ones_col, 1.0)
            nc.vector.memset(ones_row, 1.0)
            nc.vector.memset(ones_p2, 1.0)
            nc.vector.memset(shift_sb, -2.5)
            if is_bf16:
                nc.sync.dma_start(
                    out=xres_sb, in_=xres_d.rearrange("(a p) n -> p a n", p=P)
                )

            # ---- projections ----
            # qm[c, l] = sum_c' M[c, c'] x[c', l] + bw[c]
            for och in range(A):
                for lt in range(NLT):
                    ps = ps_st.tile([P, LT], f32, tag="st")
                    if proj_fp8:
                        nc.tensor.matmul(
                            ps,
                            mT_sb[:, :, och * P:(och + 1) * P],
                            x_sb[:, :, lt * LT:(lt + 1) * LT],
                            start=True, stop=True, perf_mode=DR,
                        )
                    else:
                        for a in range(A):
                            nc.tensor.matmul(
                                ps,
                                mT_sb[:, a, och * P:(och + 1) * P],
                                x_sb[:, a, lt * LT:(lt + 1) * LT],
                                start=(a == 0),
                                stop=(a == A - 1),
                            )
                    nc.vector.tensor_scalar_add(
                        out=qm_sb[:, och, lt * LT:(lt + 1) * LT],
                        in0=ps,
                        scalar1=bw_sb[:, och:och + 1],
                    )
            # vTo[j, o] = sum_c y[c, j] MoT[c, o]
            for jc in range(JC):
                ps = ps_st.tile([P, C], f32, tag="st")
                if proj_fp8:
                    nc.tensor.matmul(
                        ps,
                        y8_sb[:, :, jc * P:(jc + 1) * P],
                        moTa_sb[:, :, :],
                        start=True, stop=True, perf_mode=DR,
                    )
                else:
                    for a in range(A):
                        nc.tensor.matmul(
                            ps,
                            y_sb[:, a, jc * P:(jc + 1) * P],
                            moTa_sb[:, a, :],
                            start=(a == 0),
                            stop=(a == A - 1),
                        )
                nc.vector.tensor_copy(out=vTo_sb[:, jc, :], in_=ps)

            # ---- attention, l-tile at a time ----
            for lt in range(NLT):
                lsl = slice(lt * LT, (lt + 1) * LT)
                zq0 = ps_zq.tile([P, LT], f32, tag="zq0")
                zq1 = ps_zq.tile([P, LT], f32, tag="zq1")
                zq = (zq0, zq1)
                if val_fp8:
                    # fully fp8-DoubleRow attention: ST pairs -> one exp per
                    # pair -> DR value/denominator matmuls over jc-pairs
                    den = ps_small.tile([1, LT], f32, tag="den")
                    for jp in range(JC // 2):
                        stp = ps_st.tile([P, 2, LT], f32, tag="st")
                        for h in range(2):
                            jc = jp * 2 + h
                            nc.tensor.matmul(
                                stp[:, h, :],
                                y8_sb[:, :, jc * P:(jc + 1) * P],
                                qm_sb[:, :, lsl],
                                start=True, stop=True, perf_mode=DR,
                            )
                        e8 = epool.tile([P, 2, LT], f8)
                        # -2.5 logit shift keeps exp within fp8e4 range (max
                        # +-240); it scales numerator and denominator equally,
                        # so it cancels exactly in the softmax
                        nc.scalar.activation(
                            out=e8.rearrange("p h l -> p (h l)"),
                            in_=stp.rearrange("p h l -> p (h l)"),
                            func=mybir.ActivationFunctionType.Exp,
                            scale=float(SCALE),
                            bias=shift_sb,
                        )
                        for m in range(A):
                            nc.tensor.matmul(
                                zq[m],
                                vTo_sb[:, jp * 2:jp * 2 + 2, m * P:(m + 1) * P],
                                e8,
                                start=(jp == 0),
                                stop=(jp == JC // 2 - 1),
                                perf_mode=DR,
                            )
                        nc.tensor.matmul(
                            den,
                            ones_p2[:, :, 0:1],
                            e8,
                            start=(jp == 0),
                            stop=(jp == JC // 2 - 1),
                            perf_mode=DR,
                        )
                    r_sb = rpool.tile([1, LT], f32, tag="r")
                    nc.vector.reciprocal_approx_fast(out=r_sb, in_=den)
                    r_bf = rpool.tile([1, LT], mdt, tag="rbf")
                    nc.vector.tensor_copy(out=r_bf, in_=r_sb)
                    rbc_ps = ps_small.tile([P, LT], f32, tag="rbc")
                    nc.tensor.matmul(rbc_ps, ones_row, r_bf, start=True, stop=True)
                    rbc_sb = rpool.tile([P, LT], f32, tag="rbc")
                    nc.scalar.activation(
                        out=rbc_sb, in_=rbc_ps,
                        func=mybir.ActivationFunctionType.Copy,
                    )
                    for och in range(A):
                        o_sb = opool.tile([P, LT], f32)
                        nc.vector.tensor_mul(out=o_sb, in0=zq[och], in1=rbc_sb)
                        nc.vector.scalar_tensor_tensor(
                            out=o_sb,
                            in0=o_sb,
                            scalar=bo2_sb[:, och:och + 1],
                            in1=xres_sb[:, och, lsl],
                            op0=mybir.AluOpType.add,
                            op1=mybir.AluOpType.add,
                        )
                        nc.sync.dma_start(
                            out=out_d.rearrange("(a p) n -> p a n", p=P)[:, och, lsl],
                            in_=o_sb,
                        )
                    continue
                eacc = epool.tile([P, LT], mdt, tag="eacc")
                for jc in range(JC):
                    st = ps_st.tile([P, LT], f32, tag="st")
                    if st_fp8:
                        nc.tensor.matmul(
                            st,
                            y8_sb[:, :, jc * P:(jc + 1) * P],
                            qm_sb[:, :, lsl],
                            start=True, stop=True, perf_mode=DR,
                        )
                    else:
                        for a in range(A):
                            nc.tensor.matmul(
                                st,
                                y_sb[:, a, jc * P:(jc + 1) * P],
                                qm_sb[:, a, lsl],
                                start=(a == 0),
                                stop=(a == A - 1),
                            )
                    e_sb = epool.tile([P, LT], mdt)
                    nc.scalar.activation(
                        out=e_sb,
                        in_=st,
                        func=mybir.ActivationFunctionType.Exp,
                        scale=float(SCALE),
                    )
                    for m in range(A):
                        nc.tensor.matmul(
                            zq[m],
                            vTo_sb[:, jc, m * P:(m + 1) * P],
                            e_sb,
                            start=(jc == 0),
                            stop=(jc == JC - 1),
                        )
                    if jc == 0:
                        nc.vector.tensor_copy(out=eacc, in_=e_sb)
                    else:
                        nc.vector.tensor_add(out=eacc, in0=eacc, in1=e_sb)

                # reduce E over partitions -> denominators, then r = 1/den
                den_t = ps_st.tile([P, LT], f32, tag="st")
                den = den_t[0:1, :]
                nc.tensor.matmul(den, ones_col, eacc, start=True, stop=True)
                r_sb = rpool.tile([1, LT], f32, tag="r")
                nc.vector.reciprocal_approx_fast(out=r_sb, in_=den)
                rbc_sb = rpool.tile([P, LT], f32, tag="rbc")
                if lt == NLT - 1 and is_bf16:
                    # latency-critical final tile: broadcast r across partitions
                    # on the PE (bf16), skipping the DRAM round-trip
                    r_bf = rpool.tile([1, LT], mdt, tag="rbf")
                    nc.vector.tensor_copy(out=r_bf, in_=r_sb)
                    rbc_ps = ps_st.tile([P, LT], f32, tag="st")
                    nc.tensor.matmul(rbc_ps, ones_row, r_bf, start=True, stop=True)
                    nc.scalar.activation(
                        out=rbc_sb, in_=rbc_ps,
                        func=mybir.ActivationFunctionType.Copy,
                    )
                else:
                    # broadcast across partitions via a DRAM round-trip (off the
                    # PE/ACT critical path; overlapped by the next tile's matmuls)
                    r_dram = dpool.tile([1, LT], f32, tag="rdram")
                    nc.sync.dma_start(out=r_dram, in_=r_sb)
                    r_bcast_ap = bass.AP(
                        tensor=r_dram.tensor,
                        offset=r_dram.offset,
                        ap=[[0, P], list(r_dram.ap[-1])],
                    )
                    nc.sync.dma_start(out=rbc_sb, in_=r_bcast_ap)

                # out = zq * r + bo2 + x
                for och in range(A):
                    o_sb = opool.tile([P, LT], f32)
                    nc.vector.tensor_mul(out=o_sb, in0=zq[och], in1=rbc_sb)
                    nc.vector.scalar_tensor_tensor(
                        out=o_sb,
                        in0=o_sb,
                        scalar=bo2_sb[:, och:och + 1],
                        in1=xres_sb[:, och, lsl],
                        op0=mybir.AluOpType.add,
                        op1=mybir.AluOpType.add,
                    )
                    nc.sync.dma_start(
                        out=out_d.rearrange("(a p) n -> p a n", p=P)[:, och, lsl],
                        in_=o_sb,
                    )

    nc.compile()
    return nc


_NC_CACHE = {}


def _get_nc(key=None):
    if key is None:
        key = (MATMUL_DT, FP8_LEVEL)
    if key not in _NC_CACHE:
        _NC_CACHE[key] = build_nc(*key)
    return _NC_CACHE[key]


def make_in_maps(x, y, Wq, bq, Wk, bk, Wv, bv, Wo, bo,
                 matmul_dt_name: str = MATMUL_DT, fp8_level: int = FP8_LEVEL):
    f32 = np.float32
    f64 = np.float64
    is_bf16 = matmul_dt_name == "bfloat16"
    st_fp8 = fp8_level >= 1 and is_bf16
    proj_fp8 = fp8_level >= 2 and is_bf16
    val_fp8 = fp8_level >= 3 and is_bf16
    if is_bf16:
        import ml_dtypes

        mnp = ml_dtypes.bfloat16
        f8np = ml_dtypes.float8_e4m3
    else:
        mnp = np.float32
        f8np = None
    xnp = f8np if proj_fp8 else mnp
    xf = np.asarray(x, f32).reshape(B, C, N)
    yf = np.asarray(y, f32).reshape(B, C, N)
    Wq64, Wk64, Wv64, Wo64 = (np.asarray(w, f64) for w in (Wq, Wk, Wv, Wo))
    bq64, bv64, bo64 = (np.asarray(b, f64) for b in (bq, bv, bo))
    mT = np.ascontiguousarray((Wk64.T @ Wq64).T).astype(xnp)
    moTa = np.ascontiguousarray((Wo64 @ Wv64).T).astype(xnp)
    bw = (Wk64.T @ bq64).astype(f32)
    bo2 = (bo64 + Wo64 @ bv64).astype(f32)
    if st_fp8:
        y8 = np.clip(yf, -240, 240).astype(f8np)
    in_maps = []
    for core in range(8):
        b, h = divmod(core, 2)
        xs = np.ascontiguousarray(xf[b][:, h * NQ:(h + 1) * NQ])
        m = {
            "x": xs.astype(xnp) if xnp is not np.float32 else xs,
            "mT": mT, "moTa": moTa,
            "bw": bw, "bo2": bo2,
        }
        if not proj_fp8:
            m["y"] = yf[b].astype(mnp) if mnp is not np.float32 else yf[b]
        if st_fp8:
            m["y8"] = y8[b]
        if is_bf16:
            m["xres"] = xs
        in_maps.append(m)
    return in_maps


def kernel(x, y, Wq, bq, Wk, bk, Wv, bv, Wo, bo):
    import contextlib

    import jax

    nc = _get_nc()
    in_maps = make_in_maps(x, y, Wq, bq, Wk, bk, Wv, bv, Wo, bo)
    # Pin the axon (NeuronCore) backend: run_bass_via_pjrt uses jax.devices(),
    # which follows the ambient default platform and silently miscomputes if a
    # caller set the default to CPU.
    try:
        axon_devs = jax.devices("axon")
    except RuntimeError:
        axon_devs = None
    ctx = jax.default_device(axon_devs[0]) if axon_devs else contextlib.nullcontext()
    with ctx:
        res = bass_utils.run_bass_kernel_spmd(nc, in_maps, core_ids=list(range(8)))
    out = np.empty((B, C, N), np.float32)
    for core in range(8):
        b, h = divmod(core, 2)
        out[b][:, h * NQ:(h + 1) * NQ] = res.results[core]["out"]
    return out.reshape(B, C, H, W)


# revision 43
# speedup vs baseline: 1.0191x; 1.0191x over previous
"""Trainium2 Bass kernel for nn_Att_AdaIn (B=4, C=256, H=W=64 attention block).

Sharding: 8 cores = 4 batches x 2 query-halves. Each core holds the fused
weights, the full key/value source y[b] ([256, 4096]), and its own query
slice x[b][:, half] ([256, 2048]); it computes the full attention output for
its 2048 queries. Host gathers the 8 [256, 2048] results.

Weight fusion done on the host (in float64):
  logits: S = k^T q with q = Wq x + bq, k = Wk y + bk
        = y^T (Wk^T Wq) x + y^T (Wk^T bq) 1^T + [per-query-constant terms]
    The per-query-constant (l-only) terms are softmax-invariant and dropped.
    So with  M^T = (Wk^T Wq)^T  and  bw = Wk^T bq:   qm = M x + bw,
    ST[j,l] = sum_c y[c,j] qm[c,l].
  output: Wo (V E / den) + bo  with V = Wv y + bv 1^T
        = (Wo Wv) y E / den + Wo bv + bo
    So with MoT = (Wo Wv)^T and bo2 = bo + Wo bv, the value projection
    vTo = y^T MoT directly produces Wo-mixed values and the separate
    output projection disappears.

Per-core pipeline (layouts chosen so no on-chip transpose is needed):
  qm  = M x + bw               [c, l]      (c on partitions)
  vTo = y^T MoT                [j, 256]    (j on partitions)
  ST  = y^T qm                 [j, l]      (transposed attention scores)
  E   = exp(ST / sqrt(C))      (no max-subtraction: logits ~ N(0,1), fp32-safe)
  zq  = vTo^T E                [256, l]    unnormalized Wo-mixed output
  den = 1^T E                  [l]         softmax denominators (E summed on
                                           DVE, partition-reduced by one matmul)
  out = zq * (1/den) + bo2 + x

Dtype config via env:
  ATT_MATMUL_DT: float32 | float32r | bfloat16 (base matmul dtype)
  ATT_FP8: 0 = off (default; safest numerics, ~162 us),
           1 = fp8(e4m3) DoubleRow score matmuls (~130 us, rel err ~2e-3),
           2 = level 1 + fp8 DoubleRow qm/vTo projections,
           3 = level 2 + fp8 values/denominator with paired exps and a
               -2.5 logit shift (~127 us, rel err ~4e-3).
"""

import os
import sys

for _p in ("/root/.axon_site", "/root/.axon_site/_ro/trn_rl_repo", "/opt/trn_rl_repo"):
    if os.path.isdir(_p) and _p not in sys.path:
        sys.path.append(_p)

import numpy as np

import concourse.bass as bass
from concourse import bacc, mybir, tile
from concourse import bass_utils

B, C, H, W = 4, 256, 64, 64
N = H * W          # 4096 pixels
NQ = N // 2        # 2048 queries per core
P = 128
A = C // P         # 2 channel chunks
LT = 512           # l-tile (query) width
NLT = NQ // LT     # 4 l-tiles
JC = N // P        # 32 key chunks
SCALE = 1.0 / np.sqrt(np.float32(C))  # 1/16

MATMUL_DT = os.environ.get("ATT_MATMUL_DT", "bfloat16")
FP8_LEVEL = int(os.environ.get("ATT_FP8", "0"))


def build_nc(matmul_dt_name: str = MATMUL_DT, fp8_level: int = FP8_LEVEL):
    mdt = getattr(mybir.dt, matmul_dt_name)
    f32 = mybir.dt.float32
    f8 = mybir.dt.float8e4
    is_bf16 = mdt == mybir.dt.bfloat16
    st_fp8 = fp8_level >= 1 and is_bf16
    proj_fp8 = fp8_level >= 2 and is_bf16
    val_fp8 = fp8_level >= 3 and is_bf16
    DR = mybir.MatmulPerfMode.DoubleRow

    nc = bacc.Bacc("TRN2", target_bir_lowering=False, debug=False)

    # --- DRAM tensors ---
    xdt = f8 if proj_fp8 else mdt
    x_d = nc.dram_tensor("x", [C, NQ], xdt, kind="ExternalInput").ap()
    mT_d = nc.dram_tensor("mT", [C, C], xdt, kind="ExternalInput").ap()
    if st_fp8:
        y8_d = nc.dram_tensor("y8", [C, N], f8, kind="ExternalInput").ap()
    if not proj_fp8:
        y_d = nc.dram_tensor("y", [C, N], mdt, kind="ExternalInput").ap()
    moTa_d = nc.dram_tensor("moTa", [C, C], xdt, kind="ExternalInput").ap()
    bw_d = nc.dram_tensor("bw", [C], f32, kind="ExternalInput").ap()
    bo2_d = nc.dram_tensor("bo2", [C], f32, kind="ExternalInput").ap()
    if is_bf16:
        xres_d = nc.dram_tensor("xres", [C, NQ], f32, kind="ExternalInput").ap()
    out_d = nc.dram_tensor("out", [C, NQ], f32, kind="ExternalOutput").ap()

    qm_dt = f8 if st_fp8 else mdt

    with tile.TileContext(nc) as tc:
        with (
            tc.tile_pool(name="const", bufs=1) as const,
            tc.tile_pool(name="epool", bufs=8) as epool,
            tc.tile_pool(name="opool", bufs=3) as opool,
            tc.tile_pool(name="rpool", bufs=2) as rpool,
            tc.tile_pool(name="ps_st", bufs=2 if val_fp8 else 4, space="PSUM") as ps_st,
            tc.tile_pool(name="ps_zq", bufs=1 if val_fp8 else 2, space="PSUM") as ps_zq,
            tc.tile_pool(name="ps_small", bufs=1, space="PSUM") as ps_small,
            tc.tile_pool(name="dpool", bufs=2, space="DRAM") as dpool,
        ):
            # ---- persistent SBUF tensors ----
            x_sb = const.tile([P, A, NQ], xdt)
            mT_sb = const.tile([P, A, C], xdt)
            if st_fp8:
                y8_sb = const.tile([P, A, N], f8)
            if not proj_fp8:
                y_sb = const.tile([P, A, N], mdt)
            moTa_sb = const.tile([P, A, C], xdt)
            bw_sb = const.tile([P, A], f32)
            bo2_sb = const.tile([P, A], f32)
            ones_col = const.tile([P, 1], mdt)
            ones_row = const.tile([1, P], mdt)
            ones_p2 = const.tile([P, 2, 16], f8)
            shift_sb = const.tile([P, 1], f32)
            qm_sb = const.tile([P, A, NQ], qm_dt)
            vTo_sb = const.tile([P, JC, C], f8 if val_fp8 else mdt)
            if is_bf16:
                xres_sb = const.tile([P, A, NQ], f32)
            else:
                xres_sb = x_sb.bitcast(f32)

            # ---- loads (in order of first use; xres last, needed only at the end) ----
            xr_ = x_d.rearrange("(a p) n -> p a n", p=P)
            nc.sync.dma_start(out=x_sb[:, :, :NQ // 2], in_=xr_[:, :, :NQ // 2])
            nc.sync.dma_start(out=mT_sb, in_=mT_d.rearrange("(a p) o -> p a o", p=P))
            nc.sync.dma_start(out=bw_sb, in_=bw_d.rearrange("(a p) -> p a", p=P))
            nc.sync.dma_start(out=x_sb[:, :, NQ // 2:], in_=xr_[:, :, NQ // 2:])
            if not proj_fp8:
                yr_ = y_d.rearrange("(a p) n -> p a n", p=P)
                nc.sync.dma_start(out=y_sb[:, :, :N // 2], in_=yr_[:, :, :N // 2])
            nc.sync.dma_start(out=moTa_sb, in_=moTa_d.rearrange("(a p) o -> p a o", p=P))
            if not proj_fp8:
                nc.sync.dma_start(out=y_sb[:, :, N // 2:], in_=yr_[:, :, N // 2:])
            if st_fp8:
                y8r_ = y8_d.rearrange("(a p) n -> p a n", p=P)
                nc.sync.dma_start(out=y8_sb[:, :, :N // 2], in_=y8r_[:, :, :N // 2])
                nc.sync.dma_start(out=y8_sb[:, :, N // 2:], in_=y8r_[:, :, N // 2:])
            nc.sync.dma_start(out=bo2_sb, in_=bo2_d.rearrange("(a p) -> p a", p=P))
            nc.vector.memset(ones_col, 1.0)
            nc.vector.memset(ones_row, 1.0)
            nc.vector.memset(ones_p2, 1.0)
            nc.vector.memset(shift_sb, -2.5)
            if is_bf16:
                nc.sync.dma_start(
                    out=xres_sb, in_=xres_d.rearrange("(a p) n -> p a n", p=P)
                )

            # ---- projections ----
            # qm[c, l] = sum_c' M[c, c'] x[c', l] + bw[c]
            for och in range(A):
                for lt in range(NLT):
                    ps = ps_st.tile([P, LT], f32, tag="st")
                    if proj_fp8:
                        nc.tensor.matmul(
                            ps,
                            mT_sb[:, :, och * P:(och + 1) * P],
                            x_sb[:, :, lt * LT:(lt + 1) * LT],
                            start=True, stop=True, perf_mode=DR,
                        )
                    else:
                        for a in range(A):
                            nc.tensor.matmul(
                                ps,
                                mT_sb[:, a, och * P:(och + 1) * P],
                                x_sb[:, a, lt * LT:(lt + 1) * LT],
                                start=(a == 0),
                                stop=(a == A - 1),
                            )
                    nc.vector.tensor_scalar_add(
                        out=qm_sb[:, och, lt * LT:(lt + 1) * LT],
                        in0=ps,
                        scalar1=bw_sb[:, och:och + 1],
                    )
            # vTo[j, o] = sum_c y[c, j] MoT[c, o]
            # (for the generic path this is emitted inside the first attention
            # pass, one chunk ahead of its first use, so the PE stream never
            # stalls behind the y DMA; val_fp8 keeps the standalone loop)
            def emit_vto(jc):
                ps = ps_st.tile([P, C], f32, name="psv", tag="st")
                if proj_fp8:
                    nc.tensor.matmul(
                        ps,
                        y8_sb[:, :, jc * P:(jc + 1) * P],
                        moTa_sb[:, :, :],
                        start=True, stop=True, perf_mode=DR,
                    )
                else:
                    for a in range(A):
                        nc.tensor.matmul(
                            ps,
                            y_sb[:, a, jc * P:(jc + 1) * P],
                            moTa_sb[:, a, :],
                            start=(a == 0),
                            stop=(a == A - 1),
                        )
                nc.vector.tensor_copy(out=vTo_sb[:, jc, :], in_=ps)

            if st_fp8 or val_fp8:
                for jc in range(JC):
                    emit_vto(jc)

            # ---- attention, l-tile at a time ----
            for lt in range(NLT):
                lsl = slice(lt * LT, (lt + 1) * LT)
                zq0 = ps_zq.tile([P, LT], f32, tag="zq0")
                zq1 = ps_zq.tile([P, LT], f32, tag="zq1")
                zq = (zq0, zq1)
                if val_fp8:
                    # fully fp8-DoubleRow attention: ST pairs -> one exp per
                    # pair -> DR value/denominator matmuls over jc-pairs
                    den = ps_small.tile([1, LT], f32, tag="den")
                    for jp in range(JC // 2):
                        stp = ps_st.tile([P, 2, LT], f32, tag="st")
                        for h in range(2):
                            jc = jp * 2 + h
                            nc.tensor.matmul(
                                stp[:, h, :],
                                y8_sb[:, :, jc * P:(jc + 1) * P],
                                qm_sb[:, :, lsl],
                                start=True, stop=True, perf_mode=DR,
                            )
                        e8 = epool.tile([P, 2, LT], f8)
                        # -2.5 logit shift keeps exp within fp8e4 range (max
                        # +-240); it scales numerator and denominator equally,
                        # so it cancels exactly in the softmax
                        nc.scalar.activation(
                            out=e8.rearrange("p h l -> p (h l)"),
                            in_=stp.rearrange("p h l -> p (h l)"),
                            func=mybir.ActivationFunctionType.Exp,
                            scale=float(SCALE),
                            bias=shift_sb,
                        )
                        for m in range(A):
                            nc.tensor.matmul(
                                zq[m],
                                vTo_sb[:, jp * 2:jp * 2 + 2, m * P:(m + 1) * P],
                                e8,
                                start=(jp == 0),
                                stop=(jp == JC // 2 - 1),
                                perf_mode=DR,
                            )
                        nc.tensor.matmul(
                            den,
                            ones_p2[:, :, 0:1],
                            e8,
                            start=(jp == 0),
                            stop=(jp == JC // 2 - 1),
                            perf_mode=DR,
                        )
                    r_sb = rpool.tile([1, LT], f32, tag="r")
                    nc.vector.reciprocal_approx_fast(out=r_sb, in_=den)
                    r_bf = rpool.tile([1, LT], mdt, tag="rbf")
                    nc.vector.tensor_copy(out=r_bf, in_=r_sb)
                    rbc_ps = ps_small.tile([P, LT], f32, tag="rbc")
                    nc.tensor.matmul(rbc_ps, ones_row, r_bf, start=True, stop=True)
                    rbc_sb = rpool.tile([P, LT], f32, tag="rbc")
                    nc.scalar.activation(
                        out=rbc_sb, in_=rbc_ps,
                        func=mybir.ActivationFunctionType.Copy,
                    )
                    for och in range(A):
                        o_sb = opool.tile([P, LT], f32)
                        nc.vector.tensor_mul(out=o_sb, in0=zq[och], in1=rbc_sb)
                        nc.vector.scalar_tensor_tensor(
                            out=o_sb,
                            in0=o_sb,
                            scalar=bo2_sb[:, och:och + 1],
                            in1=xres_sb[:, och, lsl],
                            op0=mybir.AluOpType.add,
                            op1=mybir.AluOpType.add,
                        )
                        nc.sync.dma_start(
                            out=out_d.rearrange("(a p) n -> p a n", p=P)[:, och, lsl],
                            in_=o_sb,
                        )
                    continue
                eacc = epool.tile([P, LT], mdt, tag="eacc")
                for jc in range(JC):
                    if lt == 0 and not st_fp8:
                        emit_vto(jc)
                    st = ps_st.tile([P, LT], f32, tag="st")
                    if st_fp8:
                        nc.tensor.matmul(
                            st,
                            y8_sb[:, :, jc * P:(jc + 1) * P],
                            qm_sb[:, :, lsl],
                            start=True, stop=True, perf_mode=DR,
                        )
                    else:
                        for a in range(A):
                            nc.tensor.matmul(
                                st,
                                y_sb[:, a, jc * P:(jc + 1) * P],
                                qm_sb[:, a, lsl],
                                start=(a == 0),
                                stop=(a == A - 1),
                            )
                    e_sb = epool.tile([P, LT], mdt)
                    nc.scalar.activation(
                        out=e_sb,
                        in_=st,
                        func=mybir.ActivationFunctionType.Exp,
                        scale=float(SCALE),
                    )
                    for m in range(A):
                        nc.tensor.matmul(
                            zq[m],
                            vTo_sb[:, jc, m * P:(m + 1) * P],
                            e_sb,
                            start=(jc == 0),
                            stop=(jc == JC - 1),
                        )
                    if jc == 0:
                        nc.vector.tensor_copy(out=eacc, in_=e_sb)
                    else:
                        nc.vector.tensor_add(out=eacc, in0=eacc, in1=e_sb)

                # reduce E over partitions -> denominators, then r = 1/den
                den_t = ps_st.tile([P, LT], f32, tag="st")
                den = den_t[0:1, :]
                nc.tensor.matmul(den, ones_col, eacc, start=True, stop=True)
                r_sb = rpool.tile([1, LT], f32, tag="r")
                nc.vector.reciprocal_approx_fast(out=r_sb, in_=den)
                rbc_sb = rpool.tile([P, LT], f32, tag="rbc")
                if lt == NLT - 1 and is_bf16:
                    # latency-critical final tile: broadcast r across partitions
                    # on the PE (bf16), skipping the DRAM round-trip
                    r_bf = rpool.tile([1, LT], mdt, tag="rbf")
                    nc.vector.tensor_copy(out=r_bf, in_=r_sb)
                    rbc_ps = ps_st.tile([P, LT], f32, tag="st")
                    nc.tensor.matmul(rbc_ps, ones_row, r_bf, start=True, stop=True)
                    nc.scalar.activation(
                        out=rbc_sb, in_=rbc_ps,
                        func=mybir.ActivationFunctionType.Copy,
                    )
                else:
                    # broadcast across partitions via a DRAM round-trip (off the
                    # PE/ACT critical path; overlapped by the next tile's matmuls)
                    r_dram = dpool.tile([1, LT], f32, tag="rdram")
                    nc.sync.dma_start(out=r_dram, in_=r_sb)
                    r_bcast_ap = bass.AP(
                        tensor=r_dram.tensor,
                        offset=r_dram.offset,
                        ap=[[0, P], list(r_dram.ap[-1])],
                    )
                    nc.sync.dma_start(out=rbc_sb, in_=r_bcast_ap)

                # out = zq * r + bo2 + x
                for och in range(A):
                    o_sb = opool.tile([P, LT], f32)
                    nc.vector.tensor_mul(out=o_sb, in0=zq[och], in1=rbc_sb)
                    nc.vector.scalar_tensor_tensor(
                        out=o_sb,
                        in0=o_sb,
                        scalar=bo2_sb[:, och:och + 1],
                        in1=xres_sb[:, och, lsl],
                        op0=mybir.AluOpType.add,
                        op1=mybir.AluOpType.add,
                    )
                    nc.sync.dma_start(
                        out=out_d.rearrange("(a p) n -> p a n", p=P)[:, och, lsl],
                        in_=o_sb,
                    )

    nc.compile()
    return nc


_NC_CACHE = {}


def _get_nc(key=None):
    if key is None:
        key = (MATMUL_DT, FP8_LEVEL)
    if key not in _NC_CACHE:
        _NC_CACHE[key] = build_nc(*key)
    return _NC_CACHE[key]


def make_in_maps(x, y, Wq, bq, Wk, bk, Wv, bv, Wo, bo,
                 matmul_dt_name: str = MATMUL_DT, fp8_level: int = FP8_LEVEL):
    f32 = np.float32
    f64 = np.float64
    is_bf16 = matmul_dt_name == "bfloat16"
    st_fp8 = fp8_level >= 1 and is_bf16
    proj_fp8 = fp8_level >= 2 and is_bf16
    val_fp8 = fp8_level >= 3 and is_bf16
    if is_bf16:
        import ml_dtypes

        mnp = ml_dtypes.bfloat16
        f8np = ml_dtypes.float8_e4m3
    else:
        mnp = np.float32
        f8np = None
    xnp = f8np if proj_fp8 else mnp
    xf = np.asarray(x, f32).reshape(B, C, N)
    yf = np.asarray(y, f32).reshape(B, C, N)
    Wq64, Wk64, Wv64, Wo64 = (np.asarray(w, f64) for w in (Wq, Wk, Wv, Wo))
    bq64, bv64, bo64 = (np.asarray(b, f64) for b in (bq, bv, bo))
    mT = np.ascontiguousarray((Wk64.T @ Wq64).T).astype(xnp)
    moTa = np.ascontiguousarray((Wo64 @ Wv64).T).astype(xnp)
    bw = (Wk64.T @ bq64).astype(f32)
    bo2 = (bo64 + Wo64 @ bv64).astype(f32)
    if st_fp8:
        y8 = np.clip(yf, -240, 240).astype(f8np)
    in_maps = []
    for core in range(8):
        b, h = divmod(core, 2)
        xs = np.ascontiguousarray(xf[b][:, h * NQ:(h + 1) * NQ])
        m = {
            "x": xs.astype(xnp) if xnp is not np.float32 else xs,
            "mT": mT, "moTa": moTa,
            "bw": bw, "bo2": bo2,
        }
        if not proj_fp8:
            m["y"] = yf[b].astype(mnp) if mnp is not np.float32 else yf[b]
        if st_fp8:
            m["y8"] = y8[b]
        if is_bf16:
            m["xres"] = xs
        in_maps.append(m)
    return in_maps


def kernel(x, y, Wq, bq, Wk, bk, Wv, bv, Wo, bo):
    import contextlib

    import jax

    nc = _get_nc()
    in_maps = make_in_maps(x, y, Wq, bq, Wk, bk, Wv, bv, Wo, bo)
    # Pin the axon (NeuronCore) backend: run_bass_via_pjrt uses jax.devices(),
    # which follows the ambient default platform and silently miscomputes if a
    # caller set the default to CPU.
    try:
        axon_devs = jax.devices("axon")
    except RuntimeError:
        axon_devs = None
    ctx = jax.default_device(axon_devs[0]) if axon_devs else contextlib.nullcontext()
    with ctx:
        res = bass_utils.run_bass_kernel_spmd(nc, in_maps, core_ids=list(range(8)))
    out = np.empty((B, C, N), np.float32)
    for core in range(8):
        b, h = divmod(core, 2)
        out[b][:, h * NQ:(h + 1) * NQ] = res.results[core]["out"]
    return out.reshape(B, C, H, W)
